# revision 15
# baseline (speedup 1.0000x reference)
# MoE (8 experts, top-2) on 8 TRN2 NeuronCores — hidden-dim tensor-parallel.
#
# Host (numpy): router matmul + softmax + top-2 (mirrors the jax reference
# fp32 arithmetic), then packs ALL 16384 token-expert pairs sorted by
# expert into single-expert blocks of <=512 columns (near-equal widths so
# every matmul's moving stream is >=128 and the PE stationary load stays
# hidden).
# Device (per core i): processes the FULL pair stream but only a 512-wide
# slice [i*512,(i+1)*512) of the hidden dim H. All 8 experts' W1/W2 slices
# (16.8 MB bf16) are SBUF-resident, so the per-core PE work is exactly
# 16384 cols * 64 cycles = the 437us bf16 roofline with ZERO expert-
# imbalance padding. Every core runs the same SPMD program (the pair
# stream and block structure are identical across cores — only the weight
# values differ), which sidesteps the per-core-capacity problem of
# expert-parallel sharding.
#   mm1: h_slice = gelu_tanh(W1sliceT @ xT + b1slice)   [512, w] per block
#   mm2: y_partial = W2sliceT @ h_slice                 [1024, w] partial
# y partials are written out in bf16; the host sums the 8 partials in
# fp32, applies the top-2 gates, and scatter-adds into [N, D].
#
# Shapes hardcoded for B=4, S=2048, D=1024, H=4096, E=8 (spec). The block
# plan depends on the routing counts, so the Bass program is built (and
# cached) per counts-tuple.

import numpy as np
import ml_dtypes

NUM_EXPERTS = 8
TOP_K = 2
P = 128          # SBUF partitions
TB = 512         # max block width (PSUM bank holds 512 fp32)

_program_cache = {}


def _block_plan(counts):
    # Per expert: ceil(c/TB) blocks of near-equal width (>=128 keeps the
    # PE stationary load hidden). Expert 0 starts [128, 256, ...] so the
    # PE can start before the full first-block stream lands; the LAST
    # expert ends with a 128 block so the evac/DMA pipeline drains in ~1us
    # after the final matmul instead of ~10.
    E = len(counts)
    last_e = max((e for e, c in enumerate(counts) if c), default=0)
    blocks = []  # (expert, width, ordinal-within-expert)
    for e, c in enumerate(counts):
        c = int(c)
        if c == 0:
            continue
        head, tail = [], []
        if e == 0 and c > 512 + 2 * P:
            head = [P, 2 * P]
            c -= 3 * P
        if e == last_e and c > 512 + 3 * P:
            tail = [2 * P, P]
            c -= 3 * P
        nb = max(1, -(-c // TB))
        lo = c // nb
        hi = c - lo * nb
        ws = head + [lo + 1] * hi + [lo] * (nb - hi) + tail
        for j, w in enumerate(ws):
            blocks.append((e, w, j))
    return blocks


def _build_program(blocks, D, H):
    import concourse.mybir as mybir
    import concourse.tile as tile
    from concourse import bacc

    bf = mybir.dt.bfloat16
    f32 = mybir.dt.float32
    Gelu = mybir.ActivationFunctionType.Gelu_apprx_tanh

    E = NUM_EXPERTS
    HS = H // E      # per-core hidden slice (512)
    KD = D // P      # mm1 contraction chunks / mm2 output chunks (8)
    KH = HS // P     # mm1 output chunks / mm2 contraction chunks (4)
    NP = sum(w for _, w, _ in blocks)

    nc = bacc.Bacc(None, target_bir_lowering=False, debug=False)
    # Block-packed pair stream: block b occupies [KD*t0, KD*(t0+w)) so each
    # block's DMA is one contiguous 2*KD*w-byte run per partition.
    xt = nc.declare_dram_parameter("xt", [P, KD * NP], bf, isOutput=False).ap()
    w1 = nc.declare_dram_parameter("w1", [P, E, KD, HS], bf, isOutput=False).ap()
    w2 = nc.declare_dram_parameter("w2", [P, E, KH, D], bf, isOutput=False).ap()
    b1t = nc.declare_dram_parameter("b1t", [P, E, KH], f32, isOutput=False).ap()
    # y partials leave block-packed too ([P, KD*w] contiguous per block ->
    # one 2*KD*w-byte run per partition per DMA); the host unpacks
    ytr = nc.declare_dram_parameter("ytr", [P, KD * NP], bf, isOutput=True).ap()

    offs = []
    t0 = 0
    for _, w, _ in blocks:
        offs.append(t0)
        t0 += w
    NB = len(blocks)

    with tile.TileContext(nc) as tc:
        with (
            tc.tile_pool(name="weights", bufs=1) as wpool,
            tc.tile_pool(name="xin", bufs=2) as xpool,
            tc.tile_pool(name="hbuf", bufs=2) as hpool,
            tc.tile_pool(name="yout", bufs=4) as ypool,
            tc.tile_pool(name="ph", bufs=4, space="PSUM") as php,
            tc.tile_pool(name="py", bufs=4, space="PSUM") as pyp,
        ):
            # Resident weight slices. Each dma_start costs ~600 ns of
            # serialized SP-engine trigger time, so DMAs are kept COARSE:
            # one per expert weight set, one per x block, one per y block.
            w1_sb = [
                wpool.tile([P, KD, HS], bf, tag=f"w1sb{e}", name=f"w1sb{e}")
                for e in range(E)
            ]
            w2_sb = [
                wpool.tile([P, KH, D], bf, tag=f"w2sb{e}", name=f"w2sb{e}")
                for e in range(E)
            ]
            b1_sb = wpool.tile([P, E, KH], f32, tag="b1sb")

            xts = [None] * NB
            hts = [None] * NB

            def issue_x(b):
                _, w, _ = blocks[b]
                t0 = offs[b]
                x_blk = xpool.tile([P, KD, w], bf, tag="xt", name="x_blk")
                nc.sync.dma_start(
                    x_blk,
                    xt[:, KD * t0:KD * (t0 + w)].rearrange(
                        "p (k c) -> p k c", k=KD
                    ),
                )
                xts[b] = x_blk

            nblk = {}
            for e, _, j in blocks:
                nblk[e] = max(nblk.get(e, 0), j + 1)
            # prefetch ordinal: expert 0's early blocks share the startup-
            # critical DMA window, so defer its successor prefetch a bit
            pref = {e: min(3 if e == 0 else 1, nblk[e] - 1) for e in nblk}

            def emit_mm1(b):
                e, w, j = blocks[b]
                x_blk = xts[b]
                # prefetch the next expert's weights: far ahead of first
                # use (~50 us), but behind the startup-critical stream
                if j == pref[e] and e + 1 < E:
                    nc.sync.dma_start(w1_sb[e + 1], w1[:, e + 1, :, :])
                    nc.sync.dma_start(w2_sb[e + 1], w2[:, e + 1, :, :])
                hT = hpool.tile([P, KH, w], bf, tag="hT", name="hT")
                for m in range(KH):
                    ph = php.tile([P, w], f32, tag="ph", name="ph")
                    for k in range(KD):
                        nc.tensor.matmul(
                            ph,
                            w1_sb[e][:, k, m * P:(m + 1) * P],
                            x_blk[:, k, :],
                            start=(k == 0),
                            stop=(k == KD - 1),
                        )
                    nc.scalar.activation(
                        hT[:, m, :], ph, Gelu, bias=b1_sb[:, e, m:m + 1]
                    )
                hts[b] = hT

            def emit_mm2(b):
                e, w, _ = blocks[b]
                t0 = offs[b]
                hT = hts[b]
                yb = ypool.tile([P, KD, w], bf, tag="yb", name="yb")
                for d in range(KD):
                    py = pyp.tile([P, w], f32, tag="py", name="py")
                    for k in range(KH):
                        nc.tensor.matmul(
                            py,
                            w2_sb[e][:, k, d * P:(d + 1) * P],
                            hT[:, k, :],
                            start=(k == 0),
                            stop=(k == KH - 1),
                        )
                    nc.vector.tensor_copy(yb[:, d, :], py)
                nc.sync.dma_start(
                    ytr[:, KD * t0:KD * (t0 + w)].rearrange(
                        "p (d c) -> p d c", d=KD
                    ),
                    yb,
                )
                hts[b] = None

            # startup-critical DMA order: first matmuls need x0+w1s0, then
            # x1, and only then (by mm2 of block 0) w2s0. w1s0 is split
            # across two DMA rings — single-ring bandwidth (~200 GB/s) is
            # the startup limiter
            issue_x(0)
            nc.sync.dma_start(w1_sb[0][:, :KD // 2, :], w1[:, 0, :KD // 2, :])
            nc.sync.dma_start(w1_sb[0][:, KD // 2:, :], w1[:, 0, KD // 2:, :])
            issue_x(1)
            nc.sync.dma_start(w2_sb[0], w2[:, 0, :, :])
            nc.sync.dma_start(b1_sb, b1t)

            # software pipeline: PE order mm1(0), mm1(1), mm2(0), mm1(2),
            # mm2(1), ... so mm2(b) never waits on ACT's gelu evacuation
            # of its own h block; x DMAs are issued 2 blocks ahead
            emit_mm1(0)
            for b in range(NB):
                if b + 2 < NB:
                    issue_x(b + 2)
                if b + 1 < NB:
                    emit_mm1(b + 1)
                emit_mm2(b)
    nc.compile()
    return nc


def _ensure_trace_hooks():
    # bass_utils' trace path (taken when BASS_TRACE=1 is set externally)
    # imports antenv.axon_hooks, which this image lacks. Shim it (and the
    # artifact upload, which needs a bucket) only when missing, so tracing
    # degrades gracefully instead of crashing.
    import sys
    import types

    try:
        import antenv.axon_hooks  # noqa: F401
        return
    except ImportError:
        pass
    try:
        import antenv

        mod = types.ModuleType("antenv.axon_hooks")
        state = {"hook": None}
        mod.set_axon_ntff_profile_hook = lambda h: state.__setitem__("hook", h)
        mod.get_axon_ntff_profile_hook = lambda: state["hook"]
        sys.modules["antenv.axon_hooks"] = mod
        antenv.axon_hooks = mod
        try:
            from trn_agent_boot.trn_boot import _ntff_profile_via_ctypes

            mod.set_axon_ntff_profile_hook(
                _ntff_profile_via_ctypes("/opt/axon/libaxon_pjrt.so")
            )
            import concourse.bass_utils as _bu

            _orig_upload = _bu.upload_artifacts

            def _safe_upload(tmpdir):
                try:
                    return _orig_upload(tmpdir)
                except Exception:
                    return f"local:{tmpdir}"

            _bu.upload_artifacts = _safe_upload
        except Exception:
            pass
    except Exception:
        pass


def kernel(x, Wr, W1, b1, W2, b2):
    _ensure_trace_hooks()
    from concourse.bass_utils import run_bass_kernel_spmd

    bf16 = ml_dtypes.bfloat16
    B, S, D = x.shape
    E, _, H = W1.shape
    HS = H // NUM_EXPERTS
    KD = D // P
    KH = HS // P
    N = B * S
    xm = np.ascontiguousarray(x.reshape(N, D), dtype=np.float32)

    # --- host router (mirrors reference fp32 arithmetic; softmax is
    # monotonic so top-k on probs == top-k on logits, ties broken by index)
    logits = xm @ Wr
    mx = logits.max(axis=1, keepdims=True)
    ex = np.exp(logits - mx)
    probs = ex / ex.sum(axis=1, keepdims=True)
    top_i = np.argsort(-probs, axis=1, kind="stable")[:, :TOP_K]

    idx = [np.where((top_i == e).any(axis=1))[0] for e in range(E)]
    counts = [len(i) for i in idx]
    NP = int(sum(counts))

    blocks = _block_plan(counts)
    assert sum(w for _, w, _ in blocks) == NP

    # --- dispatch: pair stream sorted by expert, block-packed in SBUF
    # layout (partition-major) so every device DMA is contiguous runs
    order = np.concatenate([i for i in idx if len(i)])
    xT = np.ascontiguousarray(xm.T).astype(bf16)        # [D, N]
    xd = xT[:, order]                                   # [D, NP]
    xd3 = xd.reshape(KD, P, NP).transpose(1, 0, 2)      # [P, KD, NP]
    chunks = []
    t0 = 0
    for _, w, _ in blocks:
        chunks.append(xd3[:, :, t0:t0 + w].reshape(P, -1))
        t0 += w
    xtp = np.ascontiguousarray(np.concatenate(chunks, axis=1))  # [P, KD*NP]

    W1b = np.asarray(W1, dtype=np.float32).astype(bf16)  # [E, D, H]
    W2b = np.asarray(W2, dtype=np.float32).astype(bf16)  # [E, H, D]
    b1f = np.asarray(b1, dtype=np.float32)
    in_maps = []
    for i in range(NUM_EXPERTS):
        sl = slice(i * HS, (i + 1) * HS)
        w1s = W1b[:, :, sl]                              # [E, D, HS]
        w1p = np.ascontiguousarray(
            w1s.reshape(E, KD, P, HS).transpose(2, 0, 1, 3)
        )                                                # [P, E, KD, HS]
        w2s = W2b[:, sl, :]                              # [E, HS, D]
        w2p = np.ascontiguousarray(
            w2s.reshape(E, KH, P, D).transpose(2, 0, 1, 3)
        )                                                # [P, E, KH, D]
        b1p = np.ascontiguousarray(
            b1f[:, sl].reshape(E, KH, P).transpose(2, 0, 1)
        )                                                # [P, E, KH]
        in_maps.append({"xt": xtp, "w1": w1p, "w2": w2p, "b1t": b1p})

    key = (tuple(counts), D, H)
    if key not in _program_cache:
        _program_cache[key] = _build_program(blocks, D, H)
    nc = _program_cache[key]

    res = run_bass_kernel_spmd(nc, in_maps, core_ids=list(range(NUM_EXPERTS)))

    # --- combine: sum the 8 bf16 partials in fp32 (unpacking the block-
    # packed [P, KD*w] device layout), gate, scatter-add
    ysum = np.zeros((D, NP), dtype=np.float32)
    for i in range(NUM_EXPERTS):
        raw = np.asarray(res.results[i]["ytr"])  # [P, KD*NP] block-packed
        t0 = 0
        for _, w, _ in blocks:
            seg = raw[:, KD * t0:KD * (t0 + w)].reshape(P, KD, w)
            ysum[:, t0:t0 + w] += seg.transpose(1, 0, 2).reshape(D, w)
            t0 += w
    out = np.zeros((N, D), dtype=np.float32)
    b2f = np.asarray(b2, dtype=np.float32)
    t0 = 0
    for e in range(E):
        ne = counts[e]
        if ne == 0:
            continue
        ge = probs[idx[e], e][:, None]
        ye = ge * ysum[:, t0:t0 + ne].T
        if b2f[e].any():
            ye = ye + ge * b2f[e]
        out[idx[e]] += ye
        t0 += ne
    return out.reshape(B, S, D)
